# revision 50
# baseline (speedup 1.0000x reference)
"""Link-predictor GNN kernel for 8 TRN2 NeuronCores.

Strategy (per sharding hint): shard edges across 8 cores (data parallel),
replicate the bf16 node-embedding table + MLP weights on every core.

Edges are bucketed by (src_window, dst_window) where a window is 25000
table rows (4 windows cover 100000 nodes) so window-relative node ids fit
the int16 indices of the batched SWDGE dma_gather. Each bucket is dealt
evenly across the 8 cores (so the shared static per-bucket capacity padding
is minimal) and cut into <=CHUNK-edge chunks for pipelining.

Per chunk: one dma_gather per side (transpose=True, single_packet=False)
lands embeddings directly in X^T layout [128 dims, n_edges] in SBUF — no PE
transposes needed, and the ~1us SWDGE fixed overhead is amortized over
thousands of rows instead of the 128 an indirect_dma_start moves. The
16-partition-wrapped index arrays are loaded once in compact [16, cols]
form and replicated to the 128 partitions the gather hardware expects via
0/1-matmul broadcasts of the raw bf16 bit patterns (bit-exact, subnormals
included), cutting idx DMA traffic 8x.

MLP per 512-edge tile: h = relu(W1s^T Xs + W1d^T Xd + b1) via 4 matmuls
accumulating in PSUM; relu of h-half-0 on ACT (bias fused), half-1 on DVE
(tensor_scalar add+max). Layer 2 contracts h against W2 using h-subtiles
as the stationary operand: 2 matmuls of N=1 per 128-edge subtile writing
one PSUM column [128 edges, 1] (Ldweights is free in the cost model); a
whole chunk's logits accumulate into one PSUM tile so a single sigmoid +
one small DMA per chunk emits [128, cols] f32 results. Host inverts the
slot permutation.
"""

import sys

sys.path.insert(0, "/opt/trn_rl_repo")

import numpy as np
import ml_dtypes

from concourse import bacc, mybir, tile
from concourse.bass_utils import run_bass_kernel_spmd

BF16 = ml_dtypes.bfloat16

N_NODES = 100000
D = 128
H = 256
E_TOTAL = 600000
NCORES = 8
E_CORE = 75000
WIN = 25000                      # table-row window (< 2^15 for int16 idx)
NBUCK = 16                       # 4 src windows x 4 dst windows
CHUNK = 2944                     # max edges per gather/compute chunk
XBUFS = 4                        # gather buffer depth per side
IBUFS = 4                        # idx tile depth
HPBUFS = 2                       # PSUM h depth
L2LAG = 1                        # tiles of lag between L1 and L2 issue
HBUFS = 3                        # h sbuf tile depth
PLAN_MODE = "smalls_last"        # or "interleave"
TAPER = 6000                     # trailing slots re-split into smaller chunks
TAPER_PIECE = 1024               # taper piece size (multiple of 128)
RAMP = 0                         # 512-edge pieces peeled off the first chunk
PLBUFS = 2                       # logits PSUM depth
PBBUFS = 2                       # idx-broadcast PSUM depth
OBUFS = 3                        # sigmoid output tile depth
MIDSPLIT_N, MIDSPLIT_D = 3, 8    # mid output-store point (fraction of plan)

# Bucket capacities for the canonical setup_inputs() edge set (max count over
# the 8 cores per bucket, rounded up to 128). kernel() recomputes these from
# its actual inputs; this default only serves _build_program() callers that
# have no inputs (e.g. a standalone TimelineSim of the program).
DEFAULT_CAPS = (4736, 4736, 4736, 4864, 4736, 4736, 4736, 4736,
                4736, 4736, 4736, 4736, 4736, 4736, 4736, 4736)

LAST_RESULTS = None
_NC_CACHE: dict = {}


def _window(w):
    base = w * WIN
    return base, min(WIN, N_NODES - base)


def _bucket_of(src, dst):
    return (src // WIN) * 4 + dst // WIN


def _chunk_plan(caps):
    """Cut buckets into chunks and pick a processing order.

    Returns a list of (bucket, offset_in_bucket, size, slot_base) with
    slot_base assigned in processing order. Order: smallest chunk first
    (fast pipeline fill), small chunks spread evenly among the big ones
    (their SWDGE descriptor-gen overhead hides under big transfers), and a
    small chunk last (short drain tail).
    """
    chunks = []
    for b in range(NBUCK):
        o = 0
        while o < caps[b]:
            sz = min(CHUNK, caps[b] - o)
            chunks.append((b, o, sz))
            o += sz
    chunks.sort(key=lambda c: -c[2])
    bigs = [c for c in chunks if c[2] >= CHUNK]
    smalls = [c for c in chunks if c[2] < CHUNK]
    seq = []
    first = smalls.pop() if smalls else (bigs.pop() if bigs else None)
    if first:
        seq.append(first)
    if PLAN_MODE == "interleave" and smalls and bigs:
        last = smalls.pop() if smalls else None
        stride = -(-len(bigs) // (len(smalls) + 1))
        bi = 0
        si = 0
        while bi < len(bigs):
            seq.extend(bigs[bi : bi + stride])
            bi += stride
            if si < len(smalls):
                seq.append(smalls[si])
                si += 1
        seq.extend(smalls[si:])
        if last:
            seq.append(last)
    else:
        seq.extend(bigs)
        seq.extend(smalls)
    # ramp: peel small pieces off the front so the first gather's gen and
    # transfer are short and the pipeline fills fast
    for _ in range(RAMP):
        if seq and seq[0][2] > 512:
            b, o, sz = seq.pop(0)
            seq.insert(0, (b, o + 512, sz - 512))
            seq.insert(0, (b, o, 512))
    # taper: re-split the trailing slots into 512-edge chunks so the drain
    # after the last big transfer is short
    tail = []
    acc = 0
    while seq and acc < TAPER and seq[-1][2] > TAPER_PIECE:
        b, o, sz = seq.pop()
        for i in range(0, sz, TAPER_PIECE):
            tail.append((b, o + i, min(TAPER_PIECE, sz - i)))
        acc += sz
    seq.extend(tail)
    plan = []
    base = 0
    for b, o, sz in seq:
        plan.append((b, o, sz, base))
        base += sz
    return plan


def _build_program(caps=None):
    if caps is None:
        caps = DEFAULT_CAPS
    caps = tuple(int(c) for c in caps)
    if caps in _NC_CACHE:
        return _NC_CACHE[caps]

    dt = mybir.dt
    AF = mybir.ActivationFunctionType
    ALU = mybir.AluOpType

    EP = sum(caps)
    TOT = EP // 128
    plan = _chunk_plan(caps)
    CMAX = max(sz for _, _, sz, _ in plan)

    nc = bacc.Bacc(
        "TRN2",
        target_bir_lowering=False,
        debug=False,
        enable_asserts=False,
        num_devices=NCORES,
    )
    emd = nc.dram_tensor("emd", [N_NODES, D], dt.bfloat16, kind="ExternalInput")
    idx_d = nc.dram_tensor(
        "idx", [16, 128 + EP // 8], dt.int16, kind="ExternalInput"
    )
    w1_d = nc.dram_tensor("w1", [128, 512], dt.bfloat16, kind="ExternalInput")
    b1_d = nc.dram_tensor("b1", [128, 2], dt.float32, kind="ExternalInput")
    w2_d = nc.dram_tensor("w2", [128, 2], dt.bfloat16, kind="ExternalInput")
    b2_d = nc.dram_tensor("b2", [128, 1], dt.float32, kind="ExternalInput")
    out_d = nc.dram_tensor("out", [128, TOT], dt.float32, kind="ExternalOutput")

    with tile.TileContext(nc) as tc:
        with (
            tc.tile_pool(name="const", bufs=1) as cpool,
            tc.tile_pool(name="i", bufs=IBUFS) as ipool,
            tc.tile_pool(name="x", bufs=XBUFS) as xpool,
            tc.tile_pool(name="h", bufs=HBUFS) as hpool,
            tc.tile_pool(name="o", bufs=OBUFS) as opool,
            tc.tile_pool(name="ph", bufs=HPBUFS, space="PSUM") as php,
            tc.tile_pool(name="pl", bufs=PLBUFS, space="PSUM") as plp,
            tc.tile_pool(name="pb", bufs=PBBUFS, space="PSUM") as pbp,
        ):
            # index-path loads go first (the gather pipeline depends on
            # them); weight loads ride the Activation engine's HWDGE queue
            # one combined load: the 0/1 selection matrix (bf16-bitcast) in
            # cols 0:128, then all (16-partition-wrapped) gather indices.
            # Each chunk broadcasts its idx slice to 128 partitions via a 0/1
            # matmul on the raw bf16 bit patterns (exact, incl. subnormals)
            selidx_sb = cpool.tile([16, 128 + EP // 8], dt.int16)
            nc.sync.dma_start(selidx_sb[:, :], idx_d[:, :])
            sel_sb = selidx_sb[:, 0:128].bitcast(dt.bfloat16)
            idx16_sb = selidx_sb[:, 128:]
            osb = cpool.tile([128, TOT], dt.float32)
            w1_sb = cpool.tile([128, 512], dt.bfloat16)
            nc.scalar.dma_start(w1_sb[:, :], w1_d[:, :])
            b1_sb = cpool.tile([128, 2], dt.float32)
            nc.scalar.dma_start(b1_sb[:, :], b1_d[:, :])
            w2_sb = cpool.tile([128, 2], dt.bfloat16)
            nc.scalar.dma_start(w2_sb[:, :], w2_d[:, :])
            b2_sb = cpool.tile([128, 1], dt.float32)
            nc.scalar.dma_start(b2_sb[:, :], b2_d[:, :])

            # broadcast every chunk's indices to 128 partitions up front;
            # PE/DVE pipeline stays far ahead of the gathers consuming them
            midcol = 0
            sdis = []
            for k, (b, o, sz, base) in enumerate(plan):
                c8 = base // 8
                ibx = pbp.tile([128, CMAX // 8], dt.float32, tag="ibx")
                nc.tensor.matmul(
                    ibx[:, 0 : sz // 8], lhsT=sel_sb,
                    rhs=idx16_sb[:, c8 : c8 + sz // 8].bitcast(dt.bfloat16),
                    start=True, stop=True,
                )
                sdi = cpool.tile([128, sz // 8], dt.int16, name=f"sdi{k}")
                nc.vector.tensor_copy(
                    out=sdi[:, :].bitcast(dt.bfloat16),
                    in_=ibx[:, 0 : sz // 8],
                )
                sdis.append(sdi)

            for k, (b, o, sz, base) in enumerate(plan):
                ncols = sz // 128
                sb_, sl_ = _window(b >> 2)
                db_, dl_ = _window(b & 3)
                sdi = sdis[k]
                si = sdi[:, 0 : sz // 16]
                di = sdi[:, sz // 16 : sz // 8]
                xs = xpool.tile([128, CMAX], dt.bfloat16, tag="xs")
                xd = xpool.tile([128, CMAX], dt.bfloat16, tag="xd")
                nc.gpsimd.dma_gather(
                    xs[:, 0:sz].unsqueeze(1),
                    emd[sb_ : sb_ + sl_, :],
                    si,
                    sz,
                    sz,
                    D,
                    transpose=True,
                    single_packet=False,
                )
                nc.gpsimd.dma_gather(
                    xd[:, 0:sz].unsqueeze(1),
                    emd[db_ : db_ + dl_, :],
                    di,
                    sz,
                    sz,
                    D,
                    transpose=True,
                    single_packet=False,
                )

                lg = plp.tile([128, CMAX // 128], dt.float32, tag="lg")
                ntile = (sz + 511) // 512
                # software pipeline: L2 of tile t issues L2LAG tiles late
                hq = []
                for t in range(ntile + L2LAG):
                    if t < ntile:
                        e0 = t * 512
                        n = min(512, sz - e0)
                        h0p = php.tile([128, 512], dt.float32, tag="h0p")
                        h1p = php.tile([128, 512], dt.float32, tag="h1p")
                        nc.tensor.matmul(
                            h0p[:, 0:n], lhsT=w1_sb[:, 0:128],
                            rhs=xs[:, e0 : e0 + n], start=True, stop=False,
                        )
                        nc.tensor.matmul(
                            h0p[:, 0:n], lhsT=w1_sb[:, 256:384],
                            rhs=xd[:, e0 : e0 + n], start=False, stop=True,
                        )
                        nc.tensor.matmul(
                            h1p[:, 0:n], lhsT=w1_sb[:, 128:256],
                            rhs=xs[:, e0 : e0 + n], start=True, stop=False,
                        )
                        nc.tensor.matmul(
                            h1p[:, 0:n], lhsT=w1_sb[:, 384:512],
                            rhs=xd[:, e0 : e0 + n], start=False, stop=True,
                        )
                        h0s = hpool.tile([128, 512], dt.bfloat16, tag="h0s")
                        h1s = hpool.tile([128, 512], dt.bfloat16, tag="h1s")
                        nc.scalar.activation(
                            h0s[:, 0:n], h0p[:, 0:n], AF.Relu, bias=b1_sb[:, 0:1]
                        )
                        nc.vector.tensor_scalar(
                            h1s[:, 0:n], h1p[:, 0:n],
                            b1_sb[:, 1:2], 0.0, ALU.add, ALU.max,
                        )
                        hq.append((t, n, h0s, h1s))
                    if t >= L2LAG:
                        pt, pn, p0, p1 = hq[t - L2LAG]
                        for s in range((pn + 127) // 128):
                            ns = min(128, pn - s * 128)
                            col = pt * 4 + s
                            nc.tensor.matmul(
                                lg[0:ns, col : col + 1],
                                lhsT=p0[:, s * 128 : s * 128 + ns],
                                rhs=w2_sb[:, 0:1], start=True, stop=False,
                            )
                            nc.tensor.matmul(
                                lg[0:ns, col : col + 1],
                                lhsT=p1[:, s * 128 : s * 128 + ns],
                                rhs=w2_sb[:, 1:2], start=False, stop=True,
                            )
                nc.scalar.activation(
                    osb[:, base // 128 : base // 128 + ncols],
                    lg[:, 0:ncols], AF.Sigmoid, bias=b2_sb[:, 0:1],
                )
                if k == len(plan) * MIDSPLIT_N // MIDSPLIT_D and k < len(plan) - 2:
                    midcol = (base + sz) // 128
                    nc.sync.dma_start(out_d[:, 0:midcol], osb[:, 0:midcol])
                if k == len(plan) - 2 and midcol < (base + sz) // 128:
                    nextcol = (base + sz) // 128
                    nc.sync.dma_start(
                        out_d[:, midcol:nextcol], osb[:, midcol:nextcol]
                    )
                    midcol = nextcol
                if k == len(plan) - 1 and midcol < TOT:
                    nc.sync.dma_start(
                        out_d[:, midcol:TOT], osb[:, midcol:TOT]
                    )

    nc.compile()
    _NC_CACHE[caps] = nc
    return nc


def _wrap_idx(vals):
    """int16 [n] -> [16, n//16] wrapped in 16 partitions."""
    n = vals.shape[0]
    return np.ascontiguousarray(vals.reshape(n // 16, 16).T)


def _prepare_core(ei_core, caps, plan):
    """Bucket + chunk one core's edges into slot order.

    Returns (idx_combined, edge_of_slot): idx_combined is [16, EP//8] int16
    holding, per chunk, the wrapped src indices then the wrapped dst indices;
    edge_of_slot maps slot -> local edge id (or -1 for padding).
    """
    src = ei_core[:, 0].astype(np.int64)
    dst = ei_core[:, 1].astype(np.int64)
    bucket = _bucket_of(src, dst)
    order = np.argsort(bucket, kind="stable")
    counts = np.bincount(bucket, minlength=NBUCK)
    starts = np.zeros(NBUCK + 1, np.int64)
    np.cumsum(counts, out=starts[1:])

    EP = sum(caps)
    idx = np.zeros((16, EP // 8), np.int16)
    edge_of_slot = np.full(EP, -1, np.int64)
    for b, o, sz, base in plan:
        nb = int(counts[b])
        lo = min(o, nb)
        hi = min(o + sz, nb)
        sc = np.zeros(sz, np.int16)
        dc = np.zeros(sz, np.int16)
        if hi > lo:
            sel = order[starts[b] + lo : starts[b] + hi]
            sc[: hi - lo] = (src[sel] - (b >> 2) * WIN).astype(np.int16)
            dc[: hi - lo] = (dst[sel] - (b & 3) * WIN).astype(np.int16)
            edge_of_slot[base : base + hi - lo] = sel
        c8 = base // 8
        idx[:, c8 : c8 + sz // 16] = _wrap_idx(sc)
        idx[:, c8 + sz // 16 : c8 + sz // 8] = _wrap_idx(dc)
    return idx, edge_of_slot


def kernel(emd_all, edge_index, W1, b1, W2, b2):
    global LAST_RESULTS
    emd_bf = np.ascontiguousarray(np.asarray(emd_all, dtype=np.float32)).astype(BF16)
    ei = np.asarray(edge_index).astype(np.int64)
    W1 = np.asarray(W1, dtype=np.float32)
    W2 = np.asarray(W2, dtype=np.float32)
    b1 = np.asarray(b1, dtype=np.float32).reshape(-1)
    b2 = np.asarray(b2, dtype=np.float32).reshape(-1)

    # Re-shard edges across cores so each core gets an equal slice of every
    # bucket: per-core bucket counts become ceil(n_b/8), which minimises the
    # shared static capacity padding.
    bk_all = _bucket_of(ei[:, 0].astype(np.int64), ei[:, 1].astype(np.int64))
    gorder = np.argsort(bk_all, kind="stable")
    gcounts = np.bincount(bk_all, minlength=NBUCK)
    core_of_edge = np.empty(E_TOTAL, np.int64)
    pos = 0
    per_core_n = np.zeros(NCORES, np.int64)
    for b in range(NBUCK):
        nb = int(gcounts[b])
        sel = gorder[pos : pos + nb]
        # deal bucket b round-robin-in-blocks across cores
        q, r = divmod(nb, NCORES)
        o = 0
        for c in range(NCORES):
            take = q + (1 if c < r else 0)
            core_of_edge[sel[o : o + take]] = c
            per_core_n[c] += take
            o += take
        pos += nb
    caps_counts = np.zeros((NCORES, NBUCK), np.int64)
    edges_of_core = [np.where(core_of_edge == c)[0] for c in range(NCORES)]
    for c in range(NCORES):
        bk = bk_all[edges_of_core[c]]
        caps_counts[c] = np.bincount(bk, minlength=NBUCK)
    caps = tuple(
        int(max(128, -(-int(caps_counts[:, b].max()) // 128) * 128))
        for b in range(NBUCK)
    )
    plan = _chunk_plan(caps)

    # lhsT blocks: [src->h0, src->h1, dst->h0, dst->h1]
    w1_arr = np.concatenate(
        [W1[:D, :D], W1[:D, D:], W1[D:, :D], W1[D:, D:]], axis=1
    ).astype(BF16)
    b1_arr = np.ascontiguousarray(np.stack([b1[:128], b1[128:]], axis=1))
    w2_arr = np.ascontiguousarray(np.stack([W2[:128, 0], W2[128:, 0]], axis=1)).astype(
        BF16
    )
    b2_arr = np.full((128, 1), b2[0], np.float32)
    sel_arr = np.zeros((16, 128), np.float32)
    sel_arr[np.arange(128) % 16, np.arange(128)] = 1.0
    sel_arr = sel_arr.astype(BF16).view(np.int16)

    in_maps = []
    unshard = []
    for c in range(NCORES):
        idx, edge_of_slot = _prepare_core(ei[edges_of_core[c]], caps, plan)
        unshard.append(edge_of_slot)
        in_maps.append(
            {
                "emd": emd_bf,
                "idx": np.concatenate([sel_arr, idx], axis=1),
                "w1": w1_arr,
                "b1": b1_arr,
                "w2": w2_arr,
                "b2": b2_arr,
            }
        )

    nc = _build_program(caps)
    res = run_bass_kernel_spmd(nc, in_maps, core_ids=list(range(NCORES)))
    LAST_RESULTS = res

    y = np.empty((E_TOTAL,), np.float32)
    for c in range(NCORES):
        edge_of_slot = unshard[c]  # slot -> index into edges_of_core[c]
        out = np.asarray(res.results[c]["out"], dtype=np.float32)  # [128, TOT]
        flat = out.T.reshape(-1)  # slot-ordered
        mask = edge_of_slot >= 0
        y[edges_of_core[c][edge_of_slot[mask]]] = flat[mask]
    return y.reshape(E_TOTAL, 1)


if __name__ == "__main__":
    rng = np.random.default_rng(0)
    emd = rng.standard_normal((N_NODES, D), dtype=np.float32)
    ei = rng.integers(0, N_NODES, size=(E_TOTAL, 2)).astype(np.int32)
    W1 = rng.standard_normal((2 * D, H), dtype=np.float32) / np.sqrt(2 * D)
    W2 = rng.standard_normal((H, 1), dtype=np.float32) / np.sqrt(H)
    out = kernel(emd, ei, W1, np.zeros(H, np.float32), W2, np.zeros(1, np.float32))
    print(out.shape, out[:4, 0])
